# revision 1
# baseline (speedup 1.0000x reference)
"""Trainium2 Bass kernel for nn_ConvNet (char-CNN word encoder + sentence conv + MLP).

Model (reference):
    vw   = word_emb[words]                                  # [W, D]
    ch   = chr_emb[words_in_char].transpose -> conv1d(k=3, pad=1) -> max over L
    u    = concat([vw, wch], axis=1)                        # [W, 2D]
    r    = max over W of conv1d(u.T, k=3, pad=1)            # [2D]
    out  = tanh(r @ w1.T + b1) @ w2.T + b2                  # [1, 2]

Key algebraic trick: the char path is conv(chr_emb[ids]) with a 128-entry
char vocab.  conv o embed is linear in the one-hot encoding of the ids, so
precompute per-tap response tables  ET_k[c, :] = chr_emb[c, :] @ W_k.T  (on
device, 6 tiny matmuls) and the whole char conv collapses to a one-hot
matmul: y[:, pos] = sum_k ET_k.T @ onehot(ids[pos + k - 1]).  The one-hot is
built on device from the raw ids (DMA partition-broadcast + is_equal against
an iota column).  Word boundaries are handled by padding each word's char
sequence with char 0 (chr_emb[0] == 0, the embedding's padding_idx) so taps
never leak across words.

Sharding: data-parallel over words.  Each of the 8 cores processes 512 words
plus one halo word on each side (recomputed redundantly) so the sentence
conv needs no halo exchange; the only collective is an AllReduce(max) of the
[512]-channel sentence-conv partial max.  The tiny MLP is replicated.
"""

import sys

try:
    import concourse  # noqa: F401
except ImportError:
    sys.path.insert(0, "/opt/trn_rl_repo")

import numpy as np
import ml_dtypes

import concourse.bass as bass
import concourse.bacc as bacc
import concourse.tile as tile
from concourse import mybir
from concourse.bass_utils import run_bass_kernel_spmd

BF16 = ml_dtypes.bfloat16

CORES = 8
D = 256
L = 32
LP = L + 4          # per-word padded char stream: [0,0,chars,0,0]
CHUNK_W = 14        # words per char-conv chunk (14*36 = 504 <= 512 PSUM bank)


def _shapes(W):
    WPC = W // CORES          # real words per core
    NW = WPC + 2              # + 1 halo word each side
    TPAD = NW * LP            # padded char stream length
    IDS = TPAD + 2            # + guard col at each end
    IDS_PAD = IDS + (-IDS) % 8
    G = -(-NW // 128)         # word-gather groups of 128
    return WPC, NW, TPAD, IDS_PAD, G


def build(W):
    WPC, NW, TPAD, IDS_PAD, G = _shapes(W)
    f32 = mybir.dt.float32
    bf16 = mybir.dt.bfloat16
    i32 = mybir.dt.int32

    nc = bacc.Bacc(num_devices=CORES)

    ids = nc.declare_dram_parameter("ids", [1, IDS_PAD], bf16, isOutput=False)
    widx = nc.declare_dram_parameter("widx", [128, G], i32, isOutput=False)
    wemb = nc.declare_dram_parameter("wemb", [50000, D], f32, isOutput=False)
    cembT = nc.declare_dram_parameter("cembT", [128, 2, 128], bf16, isOutput=False)
    wkT = nc.declare_dram_parameter("wkT", [128, 3, 2, D], bf16, isOutput=False)
    cbias = nc.declare_dram_parameter("cbias", [128, 2], f32, isOutput=False)
    wsT = nc.declare_dram_parameter("wsT", [128, 3, 4, 2 * D], bf16, isOutput=False)
    bsent = nc.declare_dram_parameter("bsent", [128, 4], f32, isOutput=False)
    w1t = nc.declare_dram_parameter("w1t", [128, 4, 8, 128], bf16, isOutput=False)
    b1t = nc.declare_dram_parameter("b1t", [128, 8], f32, isOutput=False)
    w2t = nc.declare_dram_parameter("w2t", [128, 8, 2], bf16, isOutput=False)
    b2t = nc.declare_dram_parameter("b2t", [2, 1], f32, isOutput=False)
    hsc = nc.declare_dram_parameter("hsc", [128, 2], bf16, isOutput=False)
    iota = nc.declare_dram_parameter("iota", [128, 1], f32, isOutput=False)
    ident = nc.declare_dram_parameter("ident", [128, 128], f32, isOutput=False)
    out = nc.declare_dram_parameter("out", [2, 1], f32, isOutput=True)

    # chunk list for the char loop
    chunks = []
    c0 = 0
    while c0 < NW:
        chunks.append((c0, min(CHUNK_W, NW - c0)))
        c0 += CHUNK_W

    with tile.TileContext(nc) as tc:
        with (
            tc.tile_pool(name="const", bufs=1) as cpool,
            tc.tile_pool(name="gath", bufs=G) as gpool,
            tc.tile_pool(name="loop", bufs=6) as lpool,
            tc.tile_pool(name="dram", bufs=1, space="DRAM") as dpool,
        ):
            # ---- constant loads -------------------------------------------------
            widx_sb = cpool.tile([128, G], i32)
            nc.sync.dma_start(out=widx_sb[:], in_=widx[:])
            cembT_sb = cpool.tile([128, 2, 128], bf16)
            nc.scalar.dma_start(out=cembT_sb[:], in_=cembT[:])
            wkT_sb = cpool.tile([128, 3, 2, D], bf16)
            nc.scalar.dma_start(out=wkT_sb[:], in_=wkT[:])
            iota_sb = cpool.tile([128, 1], f32)
            nc.sync.dma_start(out=iota_sb[:], in_=iota[:])
            ident_sb = cpool.tile([128, 128], f32)
            nc.sync.dma_start(out=ident_sb[:], in_=ident[:])

            # ---- word-embedding gather (independent of char path) ---------------
            wrd_sb = []
            for g in range(G):
                wt = gpool.tile([128, D], f32, tag="wrd")
                nc.gpsimd.indirect_dma_start(
                    out=wt[:],
                    out_offset=None,
                    in_=wemb[:],
                    in_offset=bass.IndirectOffsetOnAxis(ap=widx_sb[:, g : g + 1], axis=0),
                )
                wrd_sb.append(wt)

            # ---- char-conv response tables  ET_k = chr_emb @ W_k.T (bf16) ------
            ET_sb = cpool.tile([128, 3, D], bf16)
            with tc.tile_pool(name="pet", bufs=3, space="PSUM") as pet:
                for k in range(3):
                    et_ps = pet.tile([128, D], f32)
                    for kc in range(2):
                        nc.tensor.matmul(
                            out=et_ps[:],
                            lhsT=cembT_sb[:, kc, :],
                            rhs=wkT_sb[:, k, kc, :],
                            start=(kc == 0),
                            stop=(kc == 1),
                        )
                    nc.any.tensor_copy(out=ET_sb[:, k, :], in_=et_ps[:])

            # ---- char path: one-hot matmul + windowed max ----------------------
            # Chunks are processed in PAIRS sharing one 4-bank PSUM tile
            # [128, 4, 512] (bank b = (pair half, channel half)), so the
            # windowed max over t needs only ONE tensor_reduce per pair.
            # wch_b[p, m, w] = max_t conv out for channel m*128+p, word w.
            # u tiles (u[0..1] = word-embedding half, filled now; u[2..3]
            # = char half, filled after the char loop)
            u = [cpool.tile([128, G * 128], bf16, tag=f"u{j}", name=f"u{j}") for j in range(4)]
            wch_b = cpool.tile([128, 2, NW], bf16, name="wch_b")
            pairs = [chunks[pi : pi + 2] for pi in range(0, len(chunks), 2)]
            with nc.named_scope("char"):
              # build every pair's one-hot FIRST so the DVE is_eq ops never
              # gate the PE matmul pipeline mid-loop
              ids_bc = cpool.tile([128, IDS_PAD], bf16, name="ids_bc")
              # broadcast the id stream in sections so the first pairs can
              # start while later sections are still streaming
              sec_js = [0, 3, 8, 13]
              sec_starts = [pairs[j][0][0] * LP for j in sec_js if j < len(pairs)]
              sec_bounds = sorted(set(sec_starts)) + [IDS_PAD]
              for a, b in zip(sec_bounds[:-1], sec_bounds[1:]):
                  nc.scalar.dma_start(
                      out=ids_bc[:, a:b],
                      in_=ids[0:1, a:b].partition_broadcast(128),
                  )
              ohs = []
              for pair in pairs:
                    c0 = pair[0][0]
                    nw_tot = sum(nw for _, nw in pair)
                    n_tot = nw_tot * LP
                    q0 = c0 * LP
                    oh = lpool.tile(
                        [128, 2 * CHUNK_W * LP + 2], bf16, tag="oh",
                        bufs=len(pairs), name=f"oh{c0}",
                    )
                    nc.vector.tensor_scalar(
                        out=oh[:, : n_tot + 2],
                        in0=ids_bc[:, q0 : q0 + n_tot + 2],
                        scalar1=iota_sb[:, 0:1],
                        scalar2=None,
                        op0=mybir.AluOpType.is_equal,
                    )
                    ohs.append(oh)
              with tc.tile_pool(name="pch", bufs=2, space="PSUM") as pch:
                for pair, oh in zip(pairs, ohs):
                    c0 = pair[0][0]
                    py = pch.tile([128, 4, 512], f32, tag="py")
                    for k in range(3):
                        for m in range(2):
                            for ci, (cc0, nw) in enumerate(pair):
                                n = nw * LP
                                off = (cc0 - c0) * LP
                                nc.tensor.matmul(
                                    out=py[:, ci * 2 + m, :n],
                                    lhsT=ET_sb[:, k, m * 128 : (m + 1) * 128],
                                    rhs=oh[:, off + k : off + k + n],
                                    start=(k == 0),
                                    stop=(k == 2),
                                )
                    # windowed max over t in [2, 34): ScalarE evacuates
                    # PSUM -> SBUF bf16, then a 5-level pairwise-max tree on
                    # DVE (bf16 SBUF, 4B-aligned slices -> 2x mode)
                    nb = 2 * len(pair)
                    nw0 = pair[0][1]
                    uniform = len(pair) == 2 and pair[1][1] == nw0
                    pv = py[:, :, : nw0 * LP].rearrange("p b (w t) -> p b w t", t=LP)
                    t5 = lpool.tile([128, 4, CHUNK_W, 32], bf16, tag="t5")
                    nc.scalar.activation(
                        out=t5[:, :nb, :nw0, :],
                        in_=pv[:, :nb, :nw0, 2:34],
                        func=mybir.ActivationFunctionType.Copy,
                    )
                    z1 = lpool.tile([128, 4, CHUNK_W, 16], bf16, tag="z1")
                    nc.vector.tensor_tensor(
                        out=z1[:, :nb, :nw0, :],
                        in0=t5[:, :nb, :nw0, 0:16],
                        in1=t5[:, :nb, :nw0, 16:32],
                        op=mybir.AluOpType.max,
                    )
                    z2 = lpool.tile([128, 4, CHUNK_W, 8], bf16, tag="z2")
                    nc.vector.tensor_tensor(
                        out=z2[:, :nb, :nw0, :],
                        in0=z1[:, :nb, :nw0, 0:8],
                        in1=z1[:, :nb, :nw0, 8:16],
                        op=mybir.AluOpType.max,
                    )
                    z3 = lpool.tile([128, 4, CHUNK_W, 4], bf16, tag="z3")
                    nc.vector.tensor_tensor(
                        out=z3[:, :nb, :nw0, :],
                        in0=z2[:, :nb, :nw0, 0:4],
                        in1=z2[:, :nb, :nw0, 4:8],
                        op=mybir.AluOpType.max,
                    )
                    z4 = lpool.tile([128, 4, CHUNK_W, 2], bf16, tag="z4")
                    nc.vector.tensor_tensor(
                        out=z4[:, :nb, :nw0, :],
                        in0=z3[:, :nb, :nw0, 0:2],
                        in1=z3[:, :nb, :nw0, 2:4],
                        op=mybir.AluOpType.max,
                    )
                    if uniform:
                        wout = wch_b[:, :, c0 : c0 + 2 * nw0].rearrange(
                            "p m (ci w) -> p ci m w", ci=2
                        ).unsqueeze(-1)
                        nc.vector.tensor_tensor(
                            out=wout,
                            in0=z4[:, :, :nw0, 0:1].rearrange(
                                "p (ci m) w t -> p ci m w t", ci=2
                            ),
                            in1=z4[:, :, :nw0, 1:2].rearrange(
                                "p (ci m) w t -> p ci m w t", ci=2
                            ),
                            op=mybir.AluOpType.max,
                        )
                    else:
                        cc0, nw = pair[0]
                        nc.vector.tensor_tensor(
                            out=wch_b[:, :, cc0 : cc0 + nw].unsqueeze(-1),
                            in0=z4[:, 0:2, :nw, 0:1],
                            in1=z4[:, 0:2, :nw, 1:2],
                            op=mybir.AluOpType.max,
                        )

            # ---- remaining constant loads (not needed until after char loop) ----
            cbias_sb = cpool.tile([128, 2], f32)
            nc.sync.dma_start(out=cbias_sb[:], in_=cbias[:])
            wsT_sb = cpool.tile([128, 3, 4, 2 * D], bf16)
            nc.sync.dma_start(out=wsT_sb[:], in_=wsT[:])
            bsent_sb = cpool.tile([128, 4], f32)
            nc.sync.dma_start(out=bsent_sb[:], in_=bsent[:])
            w1t_sb = cpool.tile([128, 4, 8, 128], bf16)
            nc.sync.dma_start(out=w1t_sb[:], in_=w1t[:])
            b1t_sb = cpool.tile([128, 8], f32)
            nc.sync.dma_start(out=b1t_sb[:], in_=b1t[:])
            w2t_sb = cpool.tile([128, 8, 2], bf16)
            nc.sync.dma_start(out=w2t_sb[:], in_=w2t[:])
            b2t_sb = cpool.tile([2, 1], f32)
            nc.sync.dma_start(out=b2t_sb[:], in_=b2t[:])
            hsc_sb = cpool.tile([128, 2], bf16)
            nc.sync.dma_start(out=hsc_sb[:], in_=hsc[:])

            # ---- assemble u^T [4][128, NW] bf16 --------------------------------
            # char half: + bias, cast to bf16
            for m in range(2):
                nc.vector.tensor_scalar(
                    out=u[2 + m][:, :NW],
                    in0=wch_b[:, m, :],
                    scalar1=cbias_sb[:, m : m + 1],
                    scalar2=None,
                    op0=mybir.AluOpType.add,
                )
            # word half: transpose gathered rows [word, ch] -> [ch, word]
            with tc.tile_pool(name="ptp", bufs=2, space="PSUM") as ptp:
                for g in range(G):
                    w = min(128, NW - g * 128)
                    for cc in range(2):
                        tp = ptp.tile([128, 128], f32, tag="tp")
                        nc.tensor.transpose(
                            out=tp[:],
                            in_=wrd_sb[g][:, cc * 128 : (cc + 1) * 128],
                            identity=ident_sb[:],
                        )
                        nc.vector.tensor_copy(
                            out=u[cc][:, g * 128 : g * 128 + w], in_=tp[:, :w]
                        )
            # halo columns: scale by 0/1 (core 0 left, core 7 right)
            for j in (2, 3):
                nc.vector.tensor_tensor(
                    out=u[j][:, 0:1], in0=u[j][:, 0:1], in1=hsc_sb[:, 0:1],
                    op=mybir.AluOpType.mult,
                )
                nc.vector.tensor_tensor(
                    out=u[j][:, NW - 1 : NW], in0=u[j][:, NW - 1 : NW],
                    in1=hsc_sb[:, 1:2], op=mybir.AluOpType.mult,
                )

            # ---- sentence conv over the word axis + local max ------------------
            rloc = cpool.tile([128, 4], f32)
            with tc.tile_pool(name="psn", bufs=4, space="PSUM") as psn:
                for m in range(4):
                    ps = psn.tile([128, WPC], f32, tag="ps")
                    first = True
                    for k in range(3):
                        for kc in range(4):
                            nc.tensor.matmul(
                                out=ps[:],
                                lhsT=wsT_sb[:, k, kc, m * 128 : (m + 1) * 128],
                                rhs=u[kc][:, k : k + WPC],
                                start=first,
                                stop=(k == 2 and kc == 3),
                            )
                            first = False
                    nc.vector.tensor_reduce(
                        out=rloc[:, m : m + 1],
                        in_=ps[:],
                        axis=mybir.AxisListType.X,
                        op=mybir.AluOpType.max,
                    )

            # ---- AllReduce(max) of the partial channel maxes -------------------
            cc_in = dpool.tile([128, 4], f32)
            cc_out = nc.dram_tensor("cc_out", [128, 4], f32, addr_space="Shared")
            nc.sync.dma_start(out=cc_in[:], in_=rloc[:])
            nc.gpsimd.collective_compute(
                "AllReduce",
                mybir.AluOpType.max,
                replica_groups=[list(range(CORES))],
                ins=[cc_in[:]],
                outs=[cc_out[:]],
            )
            rg = cpool.tile([128, 4], f32)
            nc.sync.dma_start(out=rg[:], in_=cc_out[:])
            r_sb = cpool.tile([128, 4], bf16)
            nc.vector.tensor_tensor(
                out=r_sb[:], in0=rg[:], in1=bsent_sb[:], op=mybir.AluOpType.add
            )

            # ---- MLP (replicated on every core) --------------------------------
            h_sb = cpool.tile([128, 8], bf16)
            with tc.tile_pool(name="pmlp", bufs=7, space="PSUM") as pmlp:
                for m in range(8):
                    hp = pmlp.tile([128, 1], f32, tag="hp")
                    for k in range(4):
                        nc.tensor.matmul(
                            out=hp[:],
                            lhsT=w1t_sb[:, k, m, :],
                            rhs=r_sb[:, k : k + 1],
                            start=(k == 0),
                            stop=(k == 3),
                        )
                    nc.scalar.activation(
                        out=h_sb[:, m : m + 1],
                        in_=hp[:],
                        func=mybir.ActivationFunctionType.Tanh,
                        bias=b1t_sb[:, m : m + 1],
                    )
                o_ps = pmlp.tile([2, 1], f32, tag="hp")
                for k in range(8):
                    nc.tensor.matmul(
                        out=o_ps[:],
                        lhsT=w2t_sb[:, k, :],
                        rhs=h_sb[:, k : k + 1],
                        start=(k == 0),
                        stop=(k == 7),
                    )
                o_sb = cpool.tile([2, 1], f32)
                nc.vector.tensor_tensor(
                    out=o_sb[:], in0=o_ps[:], in1=b2t_sb[:], op=mybir.AluOpType.add
                )
                nc.sync.dma_start(out=out[:], in_=o_sb[:])

    nc.finalize()
    return nc


def prep_in_maps(words, words_in_char, word_emb, chr_emb, conv_chr_w, conv_chr_b,
                 conv_sent_w, conv_sent_b, w1, b1, w2, b2):
    W = words.shape[0]
    WPC, NW, TPAD, IDS_PAD, G = _shapes(W)

    words = np.asarray(words, np.int32)
    chars = np.asarray(words_in_char, np.int32)
    word_emb = np.asarray(word_emb, np.float32)
    chr_emb = np.asarray(chr_emb, np.float32)
    conv_chr_w = np.asarray(conv_chr_w, np.float32)
    conv_chr_b = np.asarray(conv_chr_b, np.float32)
    conv_sent_w = np.asarray(conv_sent_w, np.float32)
    conv_sent_b = np.asarray(conv_sent_b, np.float32)
    w1 = np.asarray(w1, np.float32)
    b1 = np.asarray(b1, np.float32)
    w2 = np.asarray(w2, np.float32)
    b2 = np.asarray(b2, np.float32)

    # shared (layout-prepped) weights
    cembT = np.ascontiguousarray(
        chr_emb.T.reshape(2, 128, 128).transpose(1, 0, 2)
    ).astype(BF16)                                        # [p, kc, c]
    wkT = np.ascontiguousarray(
        conv_chr_w.transpose(1, 2, 0).reshape(2, 128, 3, D).transpose(1, 2, 0, 3)
    ).astype(BF16)                                        # [p, k, kc, dout]
    cbias = np.ascontiguousarray(conv_chr_b.reshape(2, 128).T).astype(np.float32)
    wsT = np.ascontiguousarray(
        conv_sent_w.transpose(1, 2, 0).reshape(4, 128, 3, 2 * D).transpose(1, 2, 0, 3)
    ).astype(BF16)                                        # [p, k, kc, c2]
    bsent = np.ascontiguousarray(conv_sent_b.reshape(4, 128).T).astype(np.float32)
    w1t = np.ascontiguousarray(
        w1.reshape(8, 128, 4, 128).transpose(3, 2, 0, 1)
    ).astype(BF16)                                  # [p, k, m, c]
    b1t = np.ascontiguousarray(b1.reshape(8, 128).T).astype(np.float32)
    w2t = np.ascontiguousarray(
        w2.T.reshape(8, 128, 2).transpose(1, 0, 2)
    ).astype(BF16)                                  # [p, k, j]
    b2t = b2.reshape(2, 1).astype(np.float32)
    iota = np.arange(128, dtype=np.float32).reshape(128, 1)
    ident = np.eye(128, dtype=np.float32)

    in_maps = []
    for c in range(CORES):
        lo = c * WPC - 1
        idxs = np.arange(lo, lo + NW)
        valid = (idxs >= 0) & (idxs < W)
        w_ext = np.where(valid, words[np.clip(idxs, 0, W - 1)], 0).astype(np.int32)
        ch_ext = np.zeros((NW, L), np.int32)
        ch_ext[valid] = chars[np.clip(idxs, 0, W - 1)[valid]]

        stream = np.zeros((NW, LP), np.int32)
        stream[:, 2 : 2 + L] = ch_ext
        ids_full = np.zeros(IDS_PAD, np.int32)
        ids_full[1 : 1 + TPAD] = stream.reshape(-1)
        ids_bf = ids_full.astype(np.float32).astype(BF16).reshape(1, IDS_PAD)

        wpad = np.zeros(G * 128, np.int32)
        wpad[:NW] = w_ext
        widx = np.ascontiguousarray(wpad.reshape(G, 128).T)

        hsc = np.ones((128, 2), np.float32)
        if c == 0:
            hsc[:, 0] = 0.0
        if c == CORES - 1:
            hsc[:, 1] = 0.0

        in_maps.append(
            dict(
                ids=ids_bf,
                widx=widx,
                wemb=word_emb,
                cembT=cembT,
                wkT=wkT,
                cbias=cbias,
                wsT=wsT,
                bsent=bsent,
                w1t=w1t,
                b1t=b1t,
                w2t=w2t,
                b2t=b2t,
                hsc=hsc.astype(BF16),
                iota=iota,
                ident=ident,
            )
        )
    return in_maps


_CACHE = {}


def _get_nc(W):
    if W not in _CACHE:
        _CACHE[W] = build(W)
    return _CACHE[W]


def run(inputs, trace=False):
    W = np.asarray(inputs["words"]).shape[0]
    nc = _get_nc(W)
    in_maps = prep_in_maps(**inputs)
    res = run_bass_kernel_spmd(nc, in_maps, list(range(CORES)), trace=trace)
    out = np.asarray(res.results[0]["out"], np.float32).reshape(1, 2)
    return out, res


def kernel(**inputs) -> np.ndarray:
    out, _ = run(inputs, trace=False)
    return out



# revision 3
# speedup vs baseline: 1.2706x; 1.2706x over previous
"""Trainium2 Bass kernel for nn_ConvNet (char-CNN word encoder + sentence conv + MLP).

Model (reference):
    vw   = word_emb[words]                                  # [W, D]
    ch   = chr_emb[words_in_char].transpose -> conv1d(k=3, pad=1) -> max over L
    u    = concat([vw, wch], axis=1)                        # [W, 2D]
    r    = max over W of conv1d(u.T, k=3, pad=1)            # [2D]
    out  = tanh(r @ w1.T + b1) @ w2.T + b2                  # [1, 2]

Char path: conv o embed is linear in the one-hot encoding of the char ids, so
precompute per-tap response tables ET_k[c, :] = chr_emb[c, :] @ W_k.T (on
device, 6 tiny matmuls, scaled x64 into fp8 range) and the char conv
collapses to a one-hot matmul.  The one-hot (fp8, exact 0/1) is built on the
HOST in a polyphase (even/odd stream) layout and DMA'd in (~2.2MB/core,
streamed in sections under the compute).

Polyphase trick: with per-word char stream [0, c0..c31, 0] (LP=34) split into
even/odd position streams E[s]=stream[2s], O[s]=stream[2s+1], the conv
outputs are
    y_even[s] = ET1[E[s]] + ET2[O[s]] + ET0[O[s-1]]
    y_odd[s]  = ET0[E[s]] + ET1[O[s]] + ET2[E[s+1]]
so each (parity, out-half) needs ONE fp8 DoubleRow matmul (k-tiles = E,O
streams, 2 taps per pass) plus ONE normal fp8 matmul: 8 passes per group
instead of 12 bf16 passes.  The windowed per-word max runs directly on the
PSUM result with a 4D strided tensor_reduce (E cols [1:17), O cols [0:16) of
each 17-col word block), eliminating the PSUM->SBUF evacuation copies.

Sharding: data-parallel over words.  Each of the 8 cores processes 512 words
plus one halo word on each side (recomputed redundantly) so the sentence
conv needs no halo exchange; the only collective is an AllReduce(max) of the
[512]-channel sentence-conv partial max.  The tiny MLP is replicated.
"""

import sys

try:
    import concourse  # noqa: F401
except ImportError:
    sys.path.insert(0, "/opt/trn_rl_repo")

import numpy as np
import ml_dtypes

import concourse.bass as bass
import concourse.bacc as bacc
import concourse.tile as tile
from concourse import mybir
from concourse.bass_utils import run_bass_kernel_spmd

BF16 = ml_dtypes.bfloat16
FP8 = ml_dtypes.float8_e4m3

CORES = 8
D = 256
L = 32
LP = 34           # per-word padded char stream: [0, chars, 0]
HL = LP // 2      # 17 cols per word per parity stream
GW = 30           # words per char-conv group (30*17 = 510 <= 512 PSUM bank)
ETS = 64.0        # fp8 scale for the ET response tables


def _shapes(W):
    WPC = W // CORES          # real words per core
    NW = WPC + 2              # + 1 halo word each side
    S = NW * HL               # parity-stream length
    SP2 = S + 2               # + guard col each end
    SP2 += (-SP2) % 16        # DoubleRow k-tile step must be 16B-aligned
    G = -(-NW // 128)         # word-gather groups of 128
    return WPC, NW, S, SP2, G


def build(W):
    WPC, NW, S, SP2, G = _shapes(W)
    f32 = mybir.dt.float32
    bf16 = mybir.dt.bfloat16
    f8 = mybir.dt.float8e4
    i32 = mybir.dt.int32

    nc = bacc.Bacc(num_devices=CORES)

    onehot = nc.declare_dram_parameter("onehot", [128, 2, SP2], f8, isOutput=False)
    widx = nc.declare_dram_parameter("widx", [128, G], i32, isOutput=False)
    wemb = nc.declare_dram_parameter("wemb", [50000, D], f32, isOutput=False)
    cembT = nc.declare_dram_parameter("cembT", [128, 2, 128], bf16, isOutput=False)
    wkTs = nc.declare_dram_parameter("wkTs", [128, 3, 2, D], bf16, isOutput=False)
    cbias = nc.declare_dram_parameter("cbias", [128, 2], f32, isOutput=False)
    wsT = nc.declare_dram_parameter("wsT", [128, 3, 4, 2 * D], bf16, isOutput=False)
    bsent = nc.declare_dram_parameter("bsent", [128, 4], f32, isOutput=False)
    w1t = nc.declare_dram_parameter("w1t", [128, 4, 8, 128], bf16, isOutput=False)
    b1t = nc.declare_dram_parameter("b1t", [128, 8], f32, isOutput=False)
    w2t = nc.declare_dram_parameter("w2t", [128, 8, 2], bf16, isOutput=False)
    b2t = nc.declare_dram_parameter("b2t", [2, 1], f32, isOutput=False)
    hsc = nc.declare_dram_parameter("hsc", [128, 2], bf16, isOutput=False)
    ident = nc.declare_dram_parameter("ident", [128, 128], f32, isOutput=False)
    out = nc.declare_dram_parameter("out", [2, 1], f32, isOutput=True)

    # char-conv groups
    groups = []
    g0 = 0
    while g0 < NW:
        groups.append((g0, min(GW, NW - g0)))
        g0 += GW

    # one-hot DMA sections (tile-dim2 col ranges), group-aligned with the
    # guard/lookahead columns folded in
    def need_col(gi):  # last tile col group gi reads, +1
        w0, nw = groups[gi]
        return 2 + (w0 + nw) * HL

    sec_groups = [0, 2, 6, 12]
    bounds = []
    for j, sg in enumerate(sec_groups):
        if sg == 0:
            bounds.append(0)
        else:
            bounds.append(need_col(sg - 1))
    bounds.append(SP2)

    with tile.TileContext(nc) as tc:
        with (
            tc.tile_pool(name="const", bufs=1) as cpool,
            tc.tile_pool(name="gath", bufs=G) as gpool,
            tc.tile_pool(name="dram", bufs=1, space="DRAM") as dpool,
        ):
            # ---- one-hot sections stream on the scalar queue -------------------
            oh_sb = cpool.tile([128, 2, SP2], f8, name="oh_sb")
            nc.scalar.dma_start(
                out=oh_sb[:, :, bounds[0] : bounds[1]],
                in_=onehot[:, :, bounds[0] : bounds[1]],
            )
            # ---- ET-table inputs on the sync queue -----------------------------
            cembT_sb = cpool.tile([128, 2, 128], bf16)
            nc.sync.dma_start(out=cembT_sb[:], in_=cembT[:])
            wkTs_sb = cpool.tile([128, 3, 2, D], bf16)
            nc.sync.dma_start(out=wkTs_sb[:], in_=wkTs[:])
            for a, b in zip(bounds[1:-1], bounds[2:]):
                nc.scalar.dma_start(out=oh_sb[:, :, a:b], in_=onehot[:, :, a:b])
            widx_sb = cpool.tile([128, G], i32)
            nc.sync.dma_start(out=widx_sb[:], in_=widx[:])
            cbias_sb = cpool.tile([128, 2], f32)
            nc.sync.dma_start(out=cbias_sb[:], in_=cbias[:])

            # ---- word-embedding gather (independent of char path) ---------------
            wrd_sb = []
            for g in range(G):
                wt = gpool.tile([128, D], f32, tag="wrd")
                nc.gpsimd.indirect_dma_start(
                    out=wt[:],
                    out_offset=None,
                    in_=wemb[:],
                    in_offset=bass.IndirectOffsetOnAxis(ap=widx_sb[:, g : g + 1], axis=0),
                )
                wrd_sb.append(wt)

            # ---- late constants (sync queue, behind the early ones) -------------
            ident_sb = cpool.tile([128, 128], f32)
            nc.sync.dma_start(out=ident_sb[:], in_=ident[:])
            wsT_sb = cpool.tile([128, 3, 4, 2 * D], bf16)
            nc.sync.dma_start(out=wsT_sb[:], in_=wsT[:])
            bsent_sb = cpool.tile([128, 4], f32)
            nc.sync.dma_start(out=bsent_sb[:], in_=bsent[:])
            w1t_sb = cpool.tile([128, 4, 8, 128], bf16)
            nc.sync.dma_start(out=w1t_sb[:], in_=w1t[:])
            b1t_sb = cpool.tile([128, 8], f32)
            nc.sync.dma_start(out=b1t_sb[:], in_=b1t[:])
            w2t_sb = cpool.tile([128, 8, 2], bf16)
            nc.sync.dma_start(out=w2t_sb[:], in_=w2t[:])
            b2t_sb = cpool.tile([2, 1], f32)
            nc.sync.dma_start(out=b2t_sb[:], in_=b2t[:])
            hsc_sb = cpool.tile([128, 2], bf16)
            nc.sync.dma_start(out=hsc_sb[:], in_=hsc[:])

            # ---- char-conv response tables  ET_k = chr_emb @ (64*W_k).T --------
            # WDR[:, par, slot, m, :]: DoubleRow weights; slot pairs with the
            # rhs parity k-tile (0=E stream, 1=O stream).
            #   par=0 (even outputs): slots (ET1, ET2);  normal tap = ET0
            #   par=1 (odd outputs):  slots (ET0, ET1);  normal tap = ET2
            WDR = cpool.tile([128, 2, 2, 2, 128], f8, name="WDR")
            WN = cpool.tile([128, 2, 2, 128], f8, name="WN")
            et_ps = []
            with tc.tile_pool(name="pet", bufs=1, space="PSUM") as pet:
                for k in range(3):
                    ps = pet.tile([128, D], f32, tag=f"et{k}")
                    for kc in range(2):
                        nc.tensor.matmul(
                            out=ps[:],
                            lhsT=cembT_sb[:, kc, :],
                            rhs=wkTs_sb[:, k, kc, :],
                            start=(kc == 0),
                            stop=(kc == 1),
                        )
                    et_ps.append(ps)
                for m in range(2):
                    sl = slice(m * 128, (m + 1) * 128)
                    nc.vector.tensor_copy(out=WDR[:, 0, 0, m, :], in_=et_ps[1][:, sl])
                    nc.vector.tensor_copy(out=WDR[:, 0, 1, m, :], in_=et_ps[2][:, sl])
                    nc.vector.tensor_copy(out=WDR[:, 1, 0, m, :], in_=et_ps[0][:, sl])
                    nc.vector.tensor_copy(out=WDR[:, 1, 1, m, :], in_=et_ps[1][:, sl])
                    nc.vector.tensor_copy(out=WN[:, 0, m, :], in_=et_ps[0][:, sl])
                    nc.vector.tensor_copy(out=WN[:, 1, m, :], in_=et_ps[2][:, sl])

            # ---- char path: polyphase one-hot matmul + windowed max ------------
            # rEb/rOb accumulate the per-word windowed maxes (x64 scale).
            rEb = cpool.tile([128, 2, NW], f32, name="rEb")
            rOb = cpool.tile([128, 2, NW], f32, name="rOb")
            with nc.named_scope("char"):
              with tc.tile_pool(name="pch", bufs=2, space="PSUM") as pch:
                for w0, nw in groups:
                    n = nw * HL
                    s0 = w0 * HL
                    py = pch.tile([128, 4, 512], f32, tag="py")
                    for par in range(2):
                        for m in range(2):
                            b = par * 2 + m
                            nc.tensor.matmul(
                                out=py[:, b, :n],
                                lhsT=WDR[:, par, :, m, :],
                                rhs=oh_sb[:, :, 1 + s0 : 1 + s0 + n],
                                start=True,
                                stop=False,
                                perf_mode=mybir.MatmulPerfMode.DoubleRow,
                            )
                            if par == 0:
                                rhs_n = oh_sb[:, 1, s0 : s0 + n]      # O[s-1]
                            else:
                                rhs_n = oh_sb[:, 0, 2 + s0 : 2 + s0 + n]  # E[s+1]
                            nc.tensor.matmul(
                                out=py[:, b, :n],
                                lhsT=WN[:, par, m, :],
                                rhs=rhs_n,
                                start=False,
                                stop=True,
                            )
                    pvE = py[:, 0:2, :n].rearrange("p b (w t) -> p b w t", t=HL)
                    nc.vector.tensor_reduce(
                        out=rEb[:, :, w0 : w0 + nw],
                        in_=pvE[:, :, :, 1:HL],
                        axis=mybir.AxisListType.X,
                        op=mybir.AluOpType.max,
                    )
                    pvO = py[:, 2:4, :n].rearrange("p b (w t) -> p b w t", t=HL)
                    nc.vector.tensor_reduce(
                        out=rOb[:, :, w0 : w0 + nw],
                        in_=pvO[:, :, :, 0 : HL - 1],
                        axis=mybir.AxisListType.X,
                        op=mybir.AluOpType.max,
                    )

            # ---- assemble u^T [4][128, NW] bf16 --------------------------------
            # u[0..1] = word-embedding halves, u[2..3] = char halves (x64 scale,
            # undone by the 1/64 folded into the char-half sentence weights).
            u = [cpool.tile([128, G * 128], bf16, tag=f"u{j}", name=f"u{j}") for j in range(4)]
            for m in range(2):
                nc.vector.tensor_tensor(
                    out=u[2 + m][:, :NW],
                    in0=rEb[:, m, :],
                    in1=rOb[:, m, :],
                    op=mybir.AluOpType.max,
                )
                nc.vector.tensor_scalar(
                    out=u[2 + m][:, :NW],
                    in0=u[2 + m][:, :NW],
                    scalar1=cbias_sb[:, m : m + 1],
                    scalar2=None,
                    op0=mybir.AluOpType.add,
                )
            # word half: transpose gathered rows [word, ch] -> [ch, word]
            with tc.tile_pool(name="ptp", bufs=2, space="PSUM") as ptp:
                for g in range(G):
                    w = min(128, NW - g * 128)
                    for cc in range(2):
                        tp = ptp.tile([128, 128], f32, tag="tp")
                        nc.tensor.transpose(
                            out=tp[:],
                            in_=wrd_sb[g][:, cc * 128 : (cc + 1) * 128],
                            identity=ident_sb[:],
                        )
                        nc.vector.tensor_copy(
                            out=u[cc][:, g * 128 : g * 128 + w], in_=tp[:, :w]
                        )
            # halo columns: scale by 0/1 (core 0 left, core 7 right)
            for j in (2, 3):
                nc.vector.tensor_tensor(
                    out=u[j][:, 0:1], in0=u[j][:, 0:1], in1=hsc_sb[:, 0:1],
                    op=mybir.AluOpType.mult,
                )
                nc.vector.tensor_tensor(
                    out=u[j][:, NW - 1 : NW], in0=u[j][:, NW - 1 : NW],
                    in1=hsc_sb[:, 1:2], op=mybir.AluOpType.mult,
                )

            # ---- sentence conv over the word axis + local max ------------------
            rloc = cpool.tile([128, 4], f32)
            with tc.tile_pool(name="psn", bufs=4, space="PSUM") as psn:
                for m in range(4):
                    ps = psn.tile([128, WPC], f32, tag="ps")
                    first = True
                    for k in range(3):
                        for kc in range(4):
                            nc.tensor.matmul(
                                out=ps[:],
                                lhsT=wsT_sb[:, k, kc, m * 128 : (m + 1) * 128],
                                rhs=u[kc][:, k : k + WPC],
                                start=first,
                                stop=(k == 2 and kc == 3),
                            )
                            first = False
                    nc.vector.tensor_reduce(
                        out=rloc[:, m : m + 1],
                        in_=ps[:],
                        axis=mybir.AxisListType.X,
                        op=mybir.AluOpType.max,
                    )

            # ---- AllReduce(max) of the partial channel maxes -------------------
            cc_in = dpool.tile([128, 4], f32)
            cc_out = nc.dram_tensor("cc_out", [128, 4], f32, addr_space="Shared")
            nc.sync.dma_start(out=cc_in[:], in_=rloc[:])
            nc.gpsimd.collective_compute(
                "AllReduce",
                mybir.AluOpType.max,
                replica_groups=[list(range(CORES))],
                ins=[cc_in[:]],
                outs=[cc_out[:]],
            )
            rg = cpool.tile([128, 4], f32)
            nc.sync.dma_start(out=rg[:], in_=cc_out[:])
            r_sb = cpool.tile([128, 4], bf16)
            nc.vector.tensor_tensor(
                out=r_sb[:], in0=rg[:], in1=bsent_sb[:], op=mybir.AluOpType.add
            )

            # ---- MLP (replicated on every core) --------------------------------
            h_sb = cpool.tile([128, 8], bf16)
            with tc.tile_pool(name="pmlp", bufs=7, space="PSUM") as pmlp:
                for m in range(8):
                    hp = pmlp.tile([128, 1], f32, tag="hp")
                    for k in range(4):
                        nc.tensor.matmul(
                            out=hp[:],
                            lhsT=w1t_sb[:, k, m, :],
                            rhs=r_sb[:, k : k + 1],
                            start=(k == 0),
                            stop=(k == 3),
                        )
                    nc.scalar.activation(
                        out=h_sb[:, m : m + 1],
                        in_=hp[:],
                        func=mybir.ActivationFunctionType.Tanh,
                        bias=b1t_sb[:, m : m + 1],
                    )
                o_ps = pmlp.tile([2, 1], f32, tag="hp")
                for k in range(8):
                    nc.tensor.matmul(
                        out=o_ps[:],
                        lhsT=w2t_sb[:, k, :],
                        rhs=h_sb[:, k : k + 1],
                        start=(k == 0),
                        stop=(k == 7),
                    )
                o_sb = cpool.tile([2, 1], f32)
                nc.vector.tensor_tensor(
                    out=o_sb[:], in0=o_ps[:], in1=b2t_sb[:], op=mybir.AluOpType.add
                )
                nc.sync.dma_start(out=out[:], in_=o_sb[:])

    nc.finalize()
    return nc


def prep_in_maps(words, words_in_char, word_emb, chr_emb, conv_chr_w, conv_chr_b,
                 conv_sent_w, conv_sent_b, w1, b1, w2, b2):
    W = words.shape[0]
    WPC, NW, S, SP2, G = _shapes(W)

    words = np.asarray(words, np.int32)
    chars = np.asarray(words_in_char, np.int32)
    word_emb = np.asarray(word_emb, np.float32)
    chr_emb = np.asarray(chr_emb, np.float32)
    conv_chr_w = np.asarray(conv_chr_w, np.float32)
    conv_chr_b = np.asarray(conv_chr_b, np.float32)
    conv_sent_w = np.asarray(conv_sent_w, np.float32)
    conv_sent_b = np.asarray(conv_sent_b, np.float32)
    w1 = np.asarray(w1, np.float32)
    b1 = np.asarray(b1, np.float32)
    w2 = np.asarray(w2, np.float32)
    b2 = np.asarray(b2, np.float32)

    # shared (layout-prepped) weights
    cembT = np.ascontiguousarray(
        chr_emb.T.reshape(2, 128, 128).transpose(1, 0, 2)
    ).astype(BF16)                                        # [p, kc, c]
    wkTs = np.ascontiguousarray(
        (ETS * conv_chr_w).transpose(1, 2, 0).reshape(2, 128, 3, D).transpose(1, 2, 0, 3)
    ).astype(BF16)                                        # [p, k, kc, dout]
    # char bias carries the x64 scale of u's char half
    cbias = np.ascontiguousarray(ETS * conv_chr_b.reshape(2, 128).T).astype(np.float32)
    # sentence conv: char-half input channels absorb the 1/64
    ws = conv_sent_w.copy()
    ws[:, D:, :] /= ETS
    wsT = np.ascontiguousarray(
        ws.transpose(1, 2, 0).reshape(4, 128, 3, 2 * D).transpose(1, 2, 0, 3)
    ).astype(BF16)                                        # [p, k, kc, c2]
    bsent = np.ascontiguousarray(conv_sent_b.reshape(4, 128).T).astype(np.float32)
    w1t = np.ascontiguousarray(
        w1.reshape(8, 128, 4, 128).transpose(3, 2, 0, 1)
    ).astype(BF16)                                  # [p, k, m, c]
    b1t = np.ascontiguousarray(b1.reshape(8, 128).T).astype(np.float32)
    w2t = np.ascontiguousarray(
        w2.T.reshape(8, 128, 2).transpose(1, 0, 2)
    ).astype(BF16)                                  # [p, k, j]
    b2t = b2.reshape(2, 1).astype(np.float32)
    ident = np.eye(128, dtype=np.float32)

    scol = np.arange(S)
    in_maps = []
    for c in range(CORES):
        lo = c * WPC - 1
        idxs = np.arange(lo, lo + NW)
        valid = (idxs >= 0) & (idxs < W)
        w_ext = np.where(valid, words[np.clip(idxs, 0, W - 1)], 0).astype(np.int32)
        ch_ext = np.zeros((NW, L), np.int32)
        ch_ext[valid] = chars[np.clip(idxs, 0, W - 1)[valid]]

        stream = np.zeros((NW, LP), np.int32)
        stream[:, 1 : 1 + L] = ch_ext
        flat = stream.reshape(-1)
        oh = np.zeros((128, 2, SP2), FP8)
        oh[flat[0::2], 0, 1 + scol] = 1.0
        oh[flat[1::2], 1, 1 + scol] = 1.0

        wpad = np.zeros(G * 128, np.int32)
        wpad[:NW] = w_ext
        widx = np.ascontiguousarray(wpad.reshape(G, 128).T)

        hsc = np.ones((128, 2), np.float32)
        if c == 0:
            hsc[:, 0] = 0.0
        if c == CORES - 1:
            hsc[:, 1] = 0.0

        in_maps.append(
            dict(
                onehot=oh,
                widx=widx,
                wemb=word_emb,
                cembT=cembT,
                wkTs=wkTs,
                cbias=cbias,
                wsT=wsT,
                bsent=bsent,
                w1t=w1t,
                b1t=b1t,
                w2t=w2t,
                b2t=b2t,
                hsc=hsc.astype(BF16),
                ident=ident,
            )
        )
    return in_maps


_CACHE = {}


def _get_nc(W):
    if W not in _CACHE:
        _CACHE[W] = build(W)
    return _CACHE[W]


def run(inputs, trace=False):
    W = np.asarray(inputs["words"]).shape[0]
    nc = _get_nc(W)
    in_maps = prep_in_maps(**inputs)
    res = run_bass_kernel_spmd(nc, in_maps, list(range(CORES)), trace=trace)
    out = np.asarray(res.results[0]["out"], np.float32).reshape(1, 2)
    return out, res


def kernel(**inputs) -> np.ndarray:
    out, _ = run(inputs, trace=False)
    return out
